# revision 28
# baseline (speedup 1.0000x reference)
"""Builder + host glue for the ViT attention kernel on 8 trn2 cores.

Reference computation (per batch b):
    qkv = x @ w_qkv.T ; q,k,v split; per head: softmax(q k^T / sqrt(dh)) v
    out = attn @ w_out.T + b_out

Sharding: data-parallel over batch (8 batches per core).

Host-side the q/k weight columns are interleaved per head-pair
(q_p0 | k_p0 | q_p1 | k_p1 | ...) so the weight DMAs stream in exactly
the order the QK projection consumes them.
"""

import numpy as np
import ml_dtypes

import concourse.bass as bass
import concourse.tile as tile
from concourse import bacc, mybir
from concourse.bass_utils import run_bass_kernel_spmd

P = 128
B, N, D = 64, 197, 768
H, DH = 12, 64
NCORES = 8
BPC = B // NCORES          # 8 batches per core
T = BPC * N                # 1576 tokens per core
KT = D // P                # 6 contraction tiles
NPAIR = H // 2             # 6 head pairs
SCALE = DH ** -0.5
VW = (DH + 1) * H          # 780: v columns incl per-head ones column
N2 = 2 * N                 # 394
JT1 = N - P                # 69: second j-tile size

BF = mybir.dt.bfloat16
F32 = mybir.dt.float32
EXP = mybir.ActivationFunctionType.Exp

T_CHUNKS = [(0, 394), (394, 394), (788, 394), (1182, 394)]


def build_nc():
    nc = bacc.Bacc(
        "TRN2", target_bir_lowering=False, debug=False, num_devices=NCORES
    )
    # Inputs are host-packed into the exact SBUF image so every input DMA
    # is a contiguous 2D slice with multi-KB per-partition lines:
    #   xTd  [128, 4*2364]: col = chunk*2364 + k*394 + t_off
    #   wqkd [128, 12*768]: col = m*768 + k*128 + c  (m use-order: q_p0,k_p0,..)
    #   wvd/wod [128, 6*768]: col = k*768 + c
    xTd = nc.dram_tensor("xTd", [P, 4 * KT * 394], BF, kind="ExternalInput").ap()
    wqkd = nc.dram_tensor("wqkd", [P, 2 * NPAIR * D], BF, kind="ExternalInput").ap()
    wvd = nc.dram_tensor("wvd", [P, KT * D], BF, kind="ExternalInput").ap()
    wod = nc.dram_tensor("wod", [P, KT * D], BF, kind="ExternalInput").ap()
    bias = nc.dram_tensor("bias", [P, KT], F32, kind="ExternalInput").ap()
    outT = nc.dram_tensor("outT", [D, T], BF, kind="ExternalOutput").ap()

    with tile.TileContext(nc) as tc:
        with (
            tc.tile_pool(name="big", bufs=1) as big,
            tc.tile_pool(name="exp", bufs=12) as sb_exp,
            tc.tile_pool(name="rec", bufs=8) as sb_rec,
            tc.tile_pool(name="bsb", bufs=8) as sb_bsb,
            tc.tile_pool(name="osb", bufs=3) as sb_osb,
            tc.tile_pool(name="ps_pj", bufs=2, space="PSUM") as ps_pj,
            tc.tile_pool(name="ps_sc", bufs=3, space="PSUM") as ps_sc,
            tc.tile_pool(name="ps_o", bufs=3, space="PSUM") as ps_o,
        ):
            # ---- persistent buffers + input DMAs -------------------------
            # DMA throughput is line-size bound (~constant lines/us per
            # queue) and issue instructions cost ~650ns of engine time, so
            # each input DMA is a contiguous slice with a 1.5-4.7KB
            # per-partition line. Only sync/scalar/gpsimd can issue DMAs.
            bias_sb = big.tile([P, KT], F32, tag="bias")

            x_all = big.tile([P, 4 * KT * 394], BF, tag="xall", name="xall")
            wqk_all = big.tile([P, 2 * NPAIR * D], BF, tag="wqkall", name="wqkall")
            wv_all = big.tile([P, KT * D], BF, tag="wvall", name="wvall")
            wo_all = big.tile([P, KT * D], BF, tag="woall", name="woall")

            CW = KT * 394  # 2364 cols per x chunk
            HW = CW // 2   # half chunk = k0-2 or k3-5

            def dma_slice(eng, dst, src, a, b):
                eng.dma_start(dst[:, a:b], src[:, a:b])

            # head schedule (~105GB/s per queue): every queue starts on
            # bytes the first QK waves need. x chunk halves alternate
            # scalar (k0-2) / gpsimd (k3-5); wqk m0,m1 lead on sync, then
            # per-pair slabs which stay ahead of the pair waves.
            for c in range(4):
                dma_slice(nc.scalar, x_all, xTd, c * CW, c * CW + HW)
                dma_slice(nc.gpsimd, x_all, xTd, c * CW + HW, (c + 1) * CW)
            dma_slice(nc.sync, wqk_all, wqkd, 0, D)          # m0
            dma_slice(nc.sync, wqk_all, wqkd, D, 2 * D)      # m1
            for p in range(1, NPAIR):
                dma_slice(nc.sync, wqk_all, wqkd, p * 2 * D, (p + 1) * 2 * D)
            nc.sync.dma_start(bias_sb[:], bias)
            # wo on gpsimd (needed ~halfway), wv split scalar/gpsimd
            dma_slice(nc.scalar, wv_all, wvd, 0, KT * D // 2)
            nc.gpsimd.dma_start(wo_all[:], wod)
            dma_slice(nc.gpsimd, wv_all, wvd, KT * D // 2, KT * D)

            def x_ap(k, t0, tl):
                c, off = divmod(t0, 394)
                base = c * CW + k * 394 + off
                return x_all[:, base : base + tl]

            def wqk_ap(k, m):
                # m: use-order index; 2p = q pair p, 2p+1 = k pair p
                c = m * D + k * P
                return wqk_all[:, c : c + P]

            def wv_ap(k, c0, cl):
                return wv_all[:, k * D + c0 : k * D + c0 + cl]

            def wo_ap(k, c0, cl):
                return wo_all[:, k * D + c0 : k * D + c0 + cl]

            # qk_sb[m]: m<6 -> q head-pair m ; m>=6 -> k head-pair m-6.
            # layout [e within pair (2 heads x 64), t global]
            qk_sb = [big.tile([P, T], BF, tag=f"qk{m}", name=f"qk{m}") for m in range(2 * NPAIR)]
            # v tiles per (batch, j-tile): [j, 12*(64+1)] with ones columns
            v_sb = [big.tile([P, VW], BF, tag=f"v{i}", name=f"v{i}") for i in range(2 * BPC)]
            for i in range(2 * BPC):
                ones_cols = v_sb[i][:].rearrange("p (h c) -> p h c", c=DH + 1)[
                    :, :, DH : DH + 1
                ]
                nc.gpsimd.memset(ones_cols, 1.0)
            # attention output, [e, t] layout, tiles per (pair, batch-pair)
            at_sb = [
                [big.tile([P, N2], BF, tag=f"at{p}_{b2}", name=f"at{p}_{b2}") for b2 in range(BPC // 2)]
                for p in range(NPAIR)
            ]

            # ---- QK projection: qkT[e, t] = (w_qk x^T) ------------------
            # wave order per pair: (q,c0),(k,c0),(q,c1),(k,c1),... so the
            # head consumes each x chunk twice before needing the next —
            # halves the startup DMA demand rate.
            qk_alt = [0]
            for p in range(NPAIR):
                for t0, tl in T_CHUNKS:
                    for tgt, m in ((p, 2 * p), (NPAIR + p, 2 * p + 1)):
                        qk_alt[0] += 1
                        if qk_alt[0] % 5 < 3:
                            psum = ps_sc.tile([P, N2], F32, tag="sc", name="qksc")[:, :tl]
                        else:
                            psum = ps_pj.tile([P, 512], F32, tag="pj", name="pj")[:, :tl]
                        for k in range(KT):
                            nc.tensor.matmul(
                                psum,
                                wqk_ap(k, m),
                                x_ap(k, t0, tl),
                                start=(k == 0),
                                stop=(k == KT - 1),
                            )
                        nc.vector.tensor_copy(
                            out=qk_sb[tgt][:, t0 : t0 + tl], in_=psum
                        )

            # ---- V projection units (filler-interleaved) ----------------
            def vproj_unit(b, jt, c0, cl):
                def emit():
                    r0 = b * N + jt * P
                    rl = P if jt == 0 else JT1
                    i = 2 * b + jt
                    psum = ps_pj.tile([P, 512], F32, tag="pj", name="pjv")[:rl, :cl]
                    for k in range(KT):
                        nc.tensor.matmul(
                            psum,
                            x_ap(k, r0, rl),
                            wv_ap(k, c0, cl),
                            start=(k == 0),
                            stop=(k == KT - 1),
                        )
                    hs = c0 // DH
                    nh = cl // DH
                    out_ap = v_sb[i][
                        :rl, (DH + 1) * hs : (DH + 1) * (hs + nh)
                    ].rearrange("p (h c) -> p h c", c=DH + 1)[:, :, 0:DH]
                    nc.scalar.copy(
                        out=out_ap,
                        in_=psum.rearrange("p (h c) -> p h c", c=DH),
                    )

                return emit

            def vproj_units(b):
                return [
                    vproj_unit(b, jt, c0, cl)
                    for jt in range(2)
                    for c0, cl in ((0, 512), (512, 256))
                ]

            # ---- out-projection units -----------------------------------
            op_alt = [0]
            out_engs = [nc.sync, nc.scalar, nc.gpsimd]

            def outproj_unit(b2, m, vec=False):
                def emit():
                    t0 = b2 * N2
                    op_alt[0] += 1
                    if op_alt[0] % 2 == 0:
                        psum = ps_sc.tile([P, N2], F32, tag="sc", name="opsc")[:, :N2]
                    else:
                        psum = ps_pj.tile([P, 512], F32, tag="pj", name="pjo")[:, :N2]
                    for k in range(KT):
                        nc.tensor.matmul(
                            psum,
                            wo_ap(k, m * P, P),
                            at_sb[k][b2][:],
                            start=(k == 0),
                            stop=(k == KT - 1),
                        )
                    osb = sb_osb.tile([P, 512], BF, tag="osb", name="osb")[:, :N2]
                    if vec:
                        # final flush: vector is idle once the last pairs
                        # are done, scalar is the serial bottleneck there
                        nc.vector.tensor_scalar_add(osb, psum, bias_sb[:, m : m + 1])
                    else:
                        nc.scalar.activation(
                            osb,
                            psum,
                            mybir.ActivationFunctionType.Identity,
                            bias=bias_sb[:, m : m + 1],
                        )
                    out_engs[op_alt[0] % 3].dma_start(
                        outT[m * P : (m + 1) * P, t0 : t0 + N2], osb
                    )

                return emit

            # ---- one attention head-pair --------------------------------
            def emit_pair(b, p):
                tb = b * N
                qT = qk_sb[p]
                kTt = qk_sb[NPAIR + p]
                expT = []
                for h in (0, 1):
                    e0 = 64 * h
                    ps_s = ps_sc.tile([P, N2], F32, tag="sc", name="sc")
                    nc.tensor.matmul(
                        ps_s[0:P, 0:N],
                        kTt[e0 : e0 + DH, tb : tb + P],
                        qT[e0 : e0 + DH, tb : tb + N],
                        start=True,
                        stop=True,
                        tile_position=(e0, 0),
                    )
                    nc.tensor.matmul(
                        ps_s[0:JT1, N:N2],
                        kTt[e0 : e0 + DH, tb + P : tb + N],
                        qT[e0 : e0 + DH, tb : tb + N],
                        start=True,
                        stop=True,
                        tile_position=(e0, 0),
                    )
                    e = sb_exp.tile([P, N2], BF, tag="expT", name="expT")
                    nc.scalar.activation(e[:], ps_s[:], EXP)
                    expT.append(e)
                pso = ps_o.tile([DH + 1, N2], F32, tag="o", name="o")
                for h in (0, 1):
                    g = 2 * p + h
                    vc = (DH + 1) * g
                    nc.tensor.matmul(
                        pso[:, N * h : N * h + N],
                        v_sb[2 * b][0:P, vc : vc + DH + 1],
                        expT[h][0:P, 0:N],
                        start=True,
                        stop=False,
                    )
                    nc.tensor.matmul(
                        pso[:, N * h : N * h + N],
                        v_sb[2 * b + 1][0:JT1, vc : vc + DH + 1],
                        expT[h][0:JT1, N:N2],
                        start=False,
                        stop=True,
                    )
                # S row -> SBUF base 0 on SCALAR (custom DVE needs base-0
                # SBUF input; scalar copy keeps it off vector's critical
                # path), then vector reciprocal, GpSimd partition broadcast,
                # normalize straight out of PSUM on vector.
                s_sb = sb_rec.tile([1, N2], F32, tag="s_sb", name="s_sb")
                nc.scalar.copy(out=s_sb[:], in_=pso[DH : DH + 1, :])
                rec = sb_rec.tile([1, N2], F32, tag="rec", name="rec")
                nc.vector.reciprocal_approx_fast(out=rec[:], in_=s_sb[:])
                bsb = sb_bsb.tile([DH, N2], F32, tag="bsb", name="bsb")
                nc.gpsimd.partition_broadcast(bsb[:], rec[:])
                for h in (0, 1):
                    nc.vector.tensor_mul(
                        out=at_sb[p][b // 2][
                            64 * h : 64 * h + DH, N * (b % 2) : N * (b % 2) + N
                        ],
                        in0=pso[0:DH, N * h : N * h + N],
                        in1=bsb[:, N * h : N * h + N],
                    )

            # ---- driver: attention with 1:1 projection filler -----------
            from collections import deque

            filler = deque()  # items: (kind, batch, emit_fn)
            for u in vproj_units(0) + vproj_units(1):
                u()
            filler.extend(("v", 2, u) for u in vproj_units(2))
            for b in range(BPC):
                # v tiles for batch b must be traced before its pairs
                for item in [it for it in filler if it[0] == "v" and it[1] <= b]:
                    filler.remove(item)
                    item[2]()
                for p in range(NPAIR):
                    emit_pair(b, p)
                    if filler:
                        filler.popleft()[2]()
                if b + 3 < BPC:
                    filler.extend(("v", b + 3, u) for u in vproj_units(b + 3))
                if b % 2 == 1:
                    filler.extend(
                        ("o", b, outproj_unit(b // 2, m, vec=(b == 7 and m % 2 == 1)))
                        for m in range(KT)
                    )
            while filler:
                filler.popleft()[2]()

    nc.compile()
    return nc


def host_in_maps(x, w_qkv, w_out, b_out):
    """Full fp32 inputs -> list of 8 per-core input dicts (bf16).

    Tensors are packed into the kernel's SBUF image (see build_nc):
      xTd  [128, 4*2364]: col = chunk*2364 + k*394 + t_off
      wqkd [128, 12*768]: col = m*768 + k*128 + c, m = q_p0,k_p0,q_p1,...
      wvd/wod [128, 6*768]: col = k*768 + c
    """
    bf16 = ml_dtypes.bfloat16
    wq = w_qkv[0:D] * SCALE
    wk = w_qkv[D : 2 * D]
    wv = w_qkv[2 * D : 3 * D]

    # wqkd: m-block = (q or k) rows [128p:128p+128]; transpose to [D, 128],
    # split D into k-tiles -> [k][128(p), 128] -> [p, k*128]
    blocks = []
    for p in range(NPAIR):
        for w in (wq, wk):
            blk = w[128 * p : 128 * (p + 1)].T  # [D, 128]
            blk = blk.reshape(KT, P, P).transpose(1, 0, 2).reshape(P, KT * P)
            blocks.append(blk)
    wqkd = np.ascontiguousarray(np.concatenate(blocks, axis=1)).astype(bf16)

    def pack_w(w):  # w [inner(D_in rows=e?), ...] -> [p, k*768]
        wT = w.T  # [D_in, D_out] with D_in = contraction
        return np.ascontiguousarray(
            wT.reshape(KT, P, D).transpose(1, 0, 2).reshape(P, KT * D)
        ).astype(bf16)

    wvd = pack_w(wv)
    wod = pack_w(w_out)
    bias = np.ascontiguousarray(b_out.reshape(KT, P).T).astype(np.float32)
    in_maps = []
    for c in range(NCORES):
        xc = x[c * BPC : (c + 1) * BPC].reshape(T, D)
        xT = xc.T  # [D, T]
        # [k, p, chunk, off] -> [p, chunk, k, off] -> [128, 4*2364]
        xTd = np.ascontiguousarray(
            xT.reshape(KT, P, 4, 394).transpose(1, 2, 0, 3).reshape(P, 4 * KT * 394)
        ).astype(bf16)
        in_maps.append(
            {"xTd": xTd, "wqkd": wqkd, "wvd": wvd, "wod": wod, "bias": bias}
        )
    return in_maps


def host_gather(results):
    """8 per-core {outT: [768, 1576] bf16} -> full [64, 197, 768] fp32."""
    out = np.empty((B, N, D), dtype=np.float32)
    for c in range(NCORES):
        oc = np.asarray(results[c]["outT"], dtype=np.float32)  # [D, T]
        out[c * BPC : (c + 1) * BPC] = oc.T.reshape(BPC, N, D)
    return out



_NC_CACHE = []


def kernel(x, w_qkv, w_out, b_out):
    """Full-input entry point: shards batch over 8 NeuronCores, runs the
    Bass kernel, gathers the full [64, 197, 768] fp32 output."""
    if not _NC_CACHE:
        _NC_CACHE.append(build_nc())
    nc = _NC_CACHE[0]
    in_maps = host_in_maps(
        np.asarray(x, dtype=np.float32),
        np.asarray(w_qkv, dtype=np.float32),
        np.asarray(w_out, dtype=np.float32),
        np.asarray(b_out, dtype=np.float32),
    )
    res = run_bass_kernel_spmd(nc, in_maps, core_ids=list(range(NCORES)))
    return host_gather(res.results)


# revision 32
# speedup vs baseline: 1.0967x; 1.0967x over previous
"""Builder + host glue for the ViT attention kernel on 8 trn2 cores.

Reference computation (per batch b):
    qkv = x @ w_qkv.T ; q,k,v split; per head: softmax(q k^T / sqrt(dh)) v
    out = attn @ w_out.T + b_out

Sharding: data-parallel over batch (8 batches per core).

Host-side the q/k weight columns are interleaved per head-pair
(q_p0 | k_p0 | q_p1 | k_p1 | ...) so the weight DMAs stream in exactly
the order the QK projection consumes them.
"""

import numpy as np
import ml_dtypes

import concourse.bass as bass
import concourse.tile as tile
from concourse import bacc, mybir
from concourse.bass_utils import run_bass_kernel_spmd

P = 128
B, N, D = 64, 197, 768
H, DH = 12, 64
NCORES = 8
BPC = B // NCORES          # 8 batches per core
T = BPC * N                # 1576 tokens per core
KT = D // P                # 6 contraction tiles
NPAIR = H // 2             # 6 head pairs
SCALE = DH ** -0.5
VW = (DH + 1) * H          # 780: v columns incl per-head ones column
N2 = 2 * N                 # 394
JT1 = N - P                # 69: second j-tile size

BF = mybir.dt.bfloat16
F32 = mybir.dt.float32
EXP = mybir.ActivationFunctionType.Exp

T_CHUNKS = [(0, 394), (394, 394), (788, 394), (1182, 394)]


def build_nc():
    nc = bacc.Bacc(
        "TRN2", target_bir_lowering=False, debug=False, num_devices=NCORES
    )
    # Inputs are host-packed into the exact SBUF image so every input DMA
    # is a contiguous 2D slice with multi-KB per-partition lines:
    #   xTd  [128, 4*2364]: col = chunk*2364 + k*394 + t_off
    #   wqkd [128, 12*768]: col = m*768 + k*128 + c  (m use-order: q_p0,k_p0,..)
    #   wvd/wod [128, 6*768]: col = k*768 + c
    xTd = nc.dram_tensor("xTd", [P, 4 * KT * 394], BF, kind="ExternalInput").ap()
    wqkd = nc.dram_tensor("wqkd", [P, 2 * NPAIR * D], BF, kind="ExternalInput").ap()
    wvd = nc.dram_tensor("wvd", [P, KT * D], BF, kind="ExternalInput").ap()
    wod = nc.dram_tensor("wod", [P, KT * D], BF, kind="ExternalInput").ap()
    bias = nc.dram_tensor("bias", [P, KT], F32, kind="ExternalInput").ap()
    outT = nc.dram_tensor("outT", [D, T], BF, kind="ExternalOutput").ap()

    with tile.TileContext(nc) as tc:
        with (
            tc.tile_pool(name="big", bufs=1) as big,
            tc.tile_pool(name="exp", bufs=12) as sb_exp,
            tc.tile_pool(name="rec", bufs=8) as sb_rec,
            tc.tile_pool(name="bsb", bufs=8) as sb_bsb,
            tc.tile_pool(name="osb", bufs=3) as sb_osb,
            tc.tile_pool(name="ps_pj", bufs=2, space="PSUM") as ps_pj,
            tc.tile_pool(name="ps_sc", bufs=3, space="PSUM") as ps_sc,
            tc.tile_pool(name="ps_o", bufs=3, space="PSUM") as ps_o,
        ):
            # ---- persistent buffers + input DMAs -------------------------
            # DMA throughput is line-size bound (~constant lines/us per
            # queue) and issue instructions cost ~650ns of engine time, so
            # each input DMA is a contiguous slice with a 1.5-4.7KB
            # per-partition line. Only sync/scalar/gpsimd can issue DMAs.
            bias_sb = big.tile([P, KT], F32, tag="bias")

            x_all = big.tile([P, 4 * KT * 394], BF, tag="xall", name="xall")
            wqk_all = big.tile([P, 2 * NPAIR * D], BF, tag="wqkall", name="wqkall")
            wv_all = big.tile([P, KT * D], BF, tag="wvall", name="wvall")
            wo_all = big.tile([P, KT * D], BF, tag="woall", name="woall")

            CW = KT * 394  # 2364 cols per x chunk
            HW = CW // 2   # half chunk = k0-2 or k3-5

            def dma_slice(eng, dst, src, a, b):
                eng.dma_start(dst[:, a:b], src[:, a:b])

            # head schedule (~105GB/s per queue): every queue starts on
            # bytes the first QK waves need. x chunk halves alternate
            # scalar (k0-2) / gpsimd (k3-5); wqk m0,m1 lead on sync, then
            # per-pair slabs which stay ahead of the pair waves.
            for c in range(4):
                dma_slice(nc.scalar, x_all, xTd, c * CW, c * CW + HW)
                dma_slice(nc.gpsimd, x_all, xTd, c * CW + HW, (c + 1) * CW)
            dma_slice(nc.sync, wqk_all, wqkd, 0, D)          # m0
            dma_slice(nc.sync, wqk_all, wqkd, D, 2 * D)      # m1
            for p in range(1, NPAIR):
                dma_slice(nc.sync, wqk_all, wqkd, p * 2 * D, (p + 1) * 2 * D)
            nc.sync.dma_start(bias_sb[:], bias)
            # wo on gpsimd (needed ~halfway), wv split scalar/gpsimd
            dma_slice(nc.scalar, wv_all, wvd, 0, KT * D // 2)
            nc.gpsimd.dma_start(wo_all[:], wod)
            dma_slice(nc.gpsimd, wv_all, wvd, KT * D // 2, KT * D)

            def x_ap(k, t0, tl):
                c, off = divmod(t0, 394)
                base = c * CW + k * 394 + off
                return x_all[:, base : base + tl]

            def wqk_ap(k, m):
                # m: use-order index; 2p = q pair p, 2p+1 = k pair p
                c = m * D + k * P
                return wqk_all[:, c : c + P]

            def wv_ap(k, c0, cl):
                return wv_all[:, k * D + c0 : k * D + c0 + cl]

            def wo_ap(k, c0, cl):
                return wo_all[:, k * D + c0 : k * D + c0 + cl]

            # qk_sb[m]: m<6 -> q head-pair m ; m>=6 -> k head-pair m-6.
            # layout [e within pair (2 heads x 64), t global]
            qk_sb = [big.tile([P, T], BF, tag=f"qk{m}", name=f"qk{m}") for m in range(2 * NPAIR)]
            # v tiles per (batch, j-tile): [j, 12*(64+1)] with ones columns
            v_sb = [big.tile([P, VW], BF, tag=f"v{i}", name=f"v{i}") for i in range(2 * BPC)]
            for i in range(2 * BPC):
                ones_cols = v_sb[i][:].rearrange("p (h c) -> p h c", c=DH + 1)[
                    :, :, DH : DH + 1
                ]
                nc.gpsimd.memset(ones_cols, 1.0)
            # attention output, [e, t] layout, tiles per (pair, batch-pair)
            at_sb = [
                [big.tile([P, N2], BF, tag=f"at{p}_{b2}", name=f"at{p}_{b2}") for b2 in range(BPC // 2)]
                for p in range(NPAIR)
            ]

            # ---- QK projection: qkT[e, t] = (w_qk x^T) ------------------
            # wave order per pair: (q,c0),(k,c0),(q,c1),(k,c1),... so the
            # head consumes each x chunk twice before needing the next —
            # halves the startup DMA demand rate.
            qk_alt = [0]
            for p in range(NPAIR):
                for t0, tl in T_CHUNKS:
                    for tgt, m in ((p, 2 * p), (NPAIR + p, 2 * p + 1)):
                        qk_alt[0] += 1
                        if qk_alt[0] % 5 < 3:
                            psum = ps_sc.tile([P, N2], F32, tag="sc", name="qksc")[:, :tl]
                        else:
                            psum = ps_pj.tile([P, 512], F32, tag="pj", name="pj")[:, :tl]
                        for k in range(KT):
                            nc.tensor.matmul(
                                psum,
                                wqk_ap(k, m),
                                x_ap(k, t0, tl),
                                start=(k == 0),
                                stop=(k == KT - 1),
                            )
                        nc.vector.tensor_copy(
                            out=qk_sb[tgt][:, t0 : t0 + tl], in_=psum
                        )

            # ---- V projection units (filler-interleaved) ----------------
            def vproj_unit(b, jt, c0, cl):
                def emit():
                    r0 = b * N + jt * P
                    rl = P if jt == 0 else JT1
                    i = 2 * b + jt
                    psum = ps_pj.tile([P, 512], F32, tag="pj", name="pjv")[:rl, :cl]
                    for k in range(KT):
                        nc.tensor.matmul(
                            psum,
                            x_ap(k, r0, rl),
                            wv_ap(k, c0, cl),
                            start=(k == 0),
                            stop=(k == KT - 1),
                        )
                    hs = c0 // DH
                    nh = cl // DH
                    out_ap = v_sb[i][
                        :rl, (DH + 1) * hs : (DH + 1) * (hs + nh)
                    ].rearrange("p (h c) -> p h c", c=DH + 1)[:, :, 0:DH]
                    nc.scalar.copy(
                        out=out_ap,
                        in_=psum.rearrange("p (h c) -> p h c", c=DH),
                    )

                return emit

            def vproj_units(b):
                return [
                    vproj_unit(b, jt, c0, cl)
                    for jt in range(2)
                    for c0, cl in ((0, 512), (512, 256))
                ]

            # ---- out-projection units -----------------------------------
            op_alt = [0]
            out_engs = [nc.sync, nc.scalar, nc.gpsimd]

            def outproj_unit(b2, m, vec=False, c0=0, cl=N2):
                def emit():
                    t0 = b2 * N2 + c0
                    op_alt[0] += 1
                    if op_alt[0] % 2 == 0:
                        psum = ps_sc.tile([P, N2], F32, tag="sc", name="opsc")[:, :cl]
                    else:
                        psum = ps_pj.tile([P, 512], F32, tag="pj", name="pjo")[:, :cl]
                    for k in range(KT):
                        nc.tensor.matmul(
                            psum,
                            wo_ap(k, m * P, P),
                            at_sb[k][b2][:, c0 : c0 + cl],
                            start=(k == 0),
                            stop=(k == KT - 1),
                        )
                    osb = sb_osb.tile([P, 512], BF, tag="osb", name="osb")[:, :cl]
                    if vec:
                        # final flush: vector is idle once the last pairs
                        # are done, scalar is the serial bottleneck there
                        nc.vector.tensor_scalar_add(osb, psum, bias_sb[:, m : m + 1])
                    else:
                        nc.scalar.activation(
                            osb,
                            psum,
                            mybir.ActivationFunctionType.Identity,
                            bias=bias_sb[:, m : m + 1],
                        )
                    out_engs[op_alt[0] % 3].dma_start(
                        outT[m * P : (m + 1) * P, t0 : t0 + cl], osb
                    )

                return emit

            # ---- one attention head-pair --------------------------------
            def emit_pair(b, p):
                tb = b * N
                qT = qk_sb[p]
                kTt = qk_sb[NPAIR + p]
                expT = []
                for h in (0, 1):
                    e0 = 64 * h
                    ps_s = ps_sc.tile([P, N2], F32, tag="sc", name="sc")
                    nc.tensor.matmul(
                        ps_s[0:P, 0:N],
                        kTt[e0 : e0 + DH, tb : tb + P],
                        qT[e0 : e0 + DH, tb : tb + N],
                        start=True,
                        stop=True,
                        tile_position=(e0, 0),
                    )
                    nc.tensor.matmul(
                        ps_s[0:JT1, N:N2],
                        kTt[e0 : e0 + DH, tb + P : tb + N],
                        qT[e0 : e0 + DH, tb : tb + N],
                        start=True,
                        stop=True,
                        tile_position=(e0, 0),
                    )
                    e = sb_exp.tile([P, N2], BF, tag="expT", name="expT")
                    nc.scalar.activation(e[:], ps_s[:], EXP)
                    expT.append(e)
                pso = ps_o.tile([DH + 1, N2], F32, tag="o", name="o")
                for h in (0, 1):
                    g = 2 * p + h
                    vc = (DH + 1) * g
                    nc.tensor.matmul(
                        pso[:, N * h : N * h + N],
                        v_sb[2 * b][0:P, vc : vc + DH + 1],
                        expT[h][0:P, 0:N],
                        start=True,
                        stop=False,
                    )
                    nc.tensor.matmul(
                        pso[:, N * h : N * h + N],
                        v_sb[2 * b + 1][0:JT1, vc : vc + DH + 1],
                        expT[h][0:JT1, N:N2],
                        start=False,
                        stop=True,
                    )
                # S row -> SBUF (base 0: custom DVE/GpSimd ops require it),
                # approx reciprocal, GpSimd partition broadcast, normalize
                # straight out of PSUM (single PSUM operand per DVE op);
                # the two muls split across vector/gpsimd to balance rates.
                s_sb = sb_rec.tile([1, N2], F32, tag="s_sb", name="s_sb")
                nc.vector.tensor_copy(out=s_sb[:], in_=pso[DH : DH + 1, :])
                rec = sb_rec.tile([1, N2], F32, tag="rec", name="rec")
                nc.vector.reciprocal_approx_fast(out=rec[:], in_=s_sb[:])
                bsb = sb_bsb.tile([DH, N2], F32, tag="bsb", name="bsb")
                nc.gpsimd.partition_broadcast(bsb[:], rec[:])
                for h in (0, 1):
                    nc.vector.tensor_mul(
                        out=at_sb[p][b // 2][
                            64 * h : 64 * h + DH, N * (b % 2) : N * (b % 2) + N
                        ],
                        in0=pso[0:DH, N * h : N * h + N],
                        in1=bsb[:, N * h : N * h + N],
                    )

            # ---- driver: attention with 1:1 projection filler -----------
            from collections import deque

            filler = deque()  # items: (kind, batch, emit_fn)
            for u in vproj_units(0) + vproj_units(1):
                u()
            filler.extend(("v", 2, u) for u in vproj_units(2))
            for b in range(BPC):
                # v tiles for batch b must be traced before its pairs
                for item in [it for it in filler if it[0] == "v" and it[1] <= b]:
                    filler.remove(item)
                    item[2]()
                for p in range(NPAIR):
                    emit_pair(b, p)
                    if filler:
                        filler.popleft()[2]()
                if b + 3 < BPC:
                    filler.extend(("v", b + 3, u) for u in vproj_units(b + 3))
                if b % 2 == 1 and b < 7:
                    filler.extend(
                        ("o", b, outproj_unit(b // 2, m)) for m in range(KT)
                    )
                if b == 6:
                    # batch-6 half of the last out-projection can flush
                    # during batch 7's pairs
                    filler.extend(
                        ("o", b, outproj_unit(3, m, c0=0, cl=N))
                        for m in range(KT)
                    )
                if b == 7:
                    filler.extend(
                        ("o", b, outproj_unit(3, m, vec=(m % 2 == 1), c0=N, cl=N))
                        for m in range(KT)
                    )
            while filler:
                filler.popleft()[2]()

    nc.compile()
    return nc


def host_in_maps(x, w_qkv, w_out, b_out):
    """Full fp32 inputs -> list of 8 per-core input dicts (bf16).

    Tensors are packed into the kernel's SBUF image (see build_nc):
      xTd  [128, 4*2364]: col = chunk*2364 + k*394 + t_off
      wqkd [128, 12*768]: col = m*768 + k*128 + c, m = q_p0,k_p0,q_p1,...
      wvd/wod [128, 6*768]: col = k*768 + c
    """
    bf16 = ml_dtypes.bfloat16
    wq = w_qkv[0:D] * SCALE
    wk = w_qkv[D : 2 * D]
    wv = w_qkv[2 * D : 3 * D]

    # wqkd: m-block = (q or k) rows [128p:128p+128]; transpose to [D, 128],
    # split D into k-tiles -> [k][128(p), 128] -> [p, k*128]
    blocks = []
    for p in range(NPAIR):
        for w in (wq, wk):
            blk = w[128 * p : 128 * (p + 1)].T  # [D, 128]
            blk = blk.reshape(KT, P, P).transpose(1, 0, 2).reshape(P, KT * P)
            blocks.append(blk)
    wqkd = np.ascontiguousarray(np.concatenate(blocks, axis=1)).astype(bf16)

    def pack_w(w):  # w [inner(D_in rows=e?), ...] -> [p, k*768]
        wT = w.T  # [D_in, D_out] with D_in = contraction
        return np.ascontiguousarray(
            wT.reshape(KT, P, D).transpose(1, 0, 2).reshape(P, KT * D)
        ).astype(bf16)

    wvd = pack_w(wv)
    wod = pack_w(w_out)
    bias = np.ascontiguousarray(b_out.reshape(KT, P).T).astype(np.float32)
    in_maps = []
    for c in range(NCORES):
        xc = x[c * BPC : (c + 1) * BPC].reshape(T, D)
        xT = xc.T  # [D, T]
        # [k, p, chunk, off] -> [p, chunk, k, off] -> [128, 4*2364]
        xTd = np.ascontiguousarray(
            xT.reshape(KT, P, 4, 394).transpose(1, 2, 0, 3).reshape(P, 4 * KT * 394)
        ).astype(bf16)
        in_maps.append(
            {"xTd": xTd, "wqkd": wqkd, "wvd": wvd, "wod": wod, "bias": bias}
        )
    return in_maps


def host_gather(results):
    """8 per-core {outT: [768, 1576] bf16} -> full [64, 197, 768] fp32."""
    out = np.empty((B, N, D), dtype=np.float32)
    for c in range(NCORES):
        oc = np.asarray(results[c]["outT"], dtype=np.float32)  # [D, T]
        out[c * BPC : (c + 1) * BPC] = oc.T.reshape(BPC, N, D)
    return out



_NC_CACHE = []


def kernel(x, w_qkv, w_out, b_out):
    """Full-input entry point: shards batch over 8 NeuronCores, runs the
    Bass kernel, gathers the full [64, 197, 768] fp32 output."""
    if not _NC_CACHE:
        _NC_CACHE.append(build_nc())
    nc = _NC_CACHE[0]
    in_maps = host_in_maps(
        np.asarray(x, dtype=np.float32),
        np.asarray(w_qkv, dtype=np.float32),
        np.asarray(w_out, dtype=np.float32),
        np.asarray(b_out, dtype=np.float32),
    )
    res = run_bass_kernel_spmd(nc, in_maps, core_ids=list(range(NCORES)))
    return host_gather(res.results)


# revision 33
# speedup vs baseline: 1.0990x; 1.0021x over previous
"""Builder + host glue for the ViT attention kernel on 8 trn2 cores.

Reference computation (per batch b):
    qkv = x @ w_qkv.T ; q,k,v split; per head: softmax(q k^T / sqrt(dh)) v
    out = attn @ w_out.T + b_out

Sharding: data-parallel over batch (8 batches per core).

Host-side the q/k weight columns are interleaved per head-pair
(q_p0 | k_p0 | q_p1 | k_p1 | ...) so the weight DMAs stream in exactly
the order the QK projection consumes them.
"""

import numpy as np
import ml_dtypes

import concourse.bass as bass
import concourse.tile as tile
from concourse import bacc, mybir
from concourse.bass_utils import run_bass_kernel_spmd

P = 128
B, N, D = 64, 197, 768
H, DH = 12, 64
NCORES = 8
BPC = B // NCORES          # 8 batches per core
T = BPC * N                # 1576 tokens per core
KT = D // P                # 6 contraction tiles
NPAIR = H // 2             # 6 head pairs
SCALE = DH ** -0.5
VW = (DH + 1) * H          # 780: v columns incl per-head ones column
N2 = 2 * N                 # 394
JT1 = N - P                # 69: second j-tile size

BF = mybir.dt.bfloat16
F32 = mybir.dt.float32
EXP = mybir.ActivationFunctionType.Exp

T_CHUNKS = [(0, 394), (394, 394), (788, 394), (1182, 394)]


def build_nc():
    nc = bacc.Bacc(
        "TRN2", target_bir_lowering=False, debug=False, num_devices=NCORES
    )
    # Inputs are host-packed into the exact SBUF image so every input DMA
    # is a contiguous 2D slice with multi-KB per-partition lines:
    #   xTd  [128, 4*2364]: col = chunk*2364 + k*394 + t_off
    #   wqkd [128, 12*768]: col = m*768 + k*128 + c  (m use-order: q_p0,k_p0,..)
    #   wvd/wod [128, 6*768]: col = k*768 + c
    xTd = nc.dram_tensor("xTd", [P, 4 * KT * 394], BF, kind="ExternalInput").ap()
    wqkd = nc.dram_tensor("wqkd", [P, 2 * NPAIR * D], BF, kind="ExternalInput").ap()
    wvd = nc.dram_tensor("wvd", [P, KT * D], BF, kind="ExternalInput").ap()
    wod = nc.dram_tensor("wod", [P, KT * D], BF, kind="ExternalInput").ap()
    bias = nc.dram_tensor("bias", [P, KT], F32, kind="ExternalInput").ap()
    outT = nc.dram_tensor("outT", [D, T], BF, kind="ExternalOutput").ap()

    with tile.TileContext(nc) as tc:
        with (
            tc.tile_pool(name="big", bufs=1) as big,
            tc.tile_pool(name="exp", bufs=12) as sb_exp,
            tc.tile_pool(name="rec", bufs=8) as sb_rec,
            tc.tile_pool(name="bsb", bufs=8) as sb_bsb,
            tc.tile_pool(name="osb", bufs=3) as sb_osb,
            tc.tile_pool(name="ps_pj", bufs=2, space="PSUM") as ps_pj,
            tc.tile_pool(name="ps_sc", bufs=3, space="PSUM") as ps_sc,
            tc.tile_pool(name="ps_o", bufs=3, space="PSUM") as ps_o,
        ):
            # ---- persistent buffers + input DMAs -------------------------
            # DMA throughput is line-size bound (~constant lines/us per
            # queue) and issue instructions cost ~650ns of engine time, so
            # each input DMA is a contiguous slice with a 1.5-4.7KB
            # per-partition line. Only sync/scalar/gpsimd can issue DMAs.
            bias_sb = big.tile([P, KT], F32, tag="bias")

            x_all = big.tile([P, 4 * KT * 394], BF, tag="xall", name="xall")
            wqk_all = big.tile([P, 2 * NPAIR * D], BF, tag="wqkall", name="wqkall")
            wv_all = big.tile([P, KT * D], BF, tag="wvall", name="wvall")
            wo_all = big.tile([P, KT * D], BF, tag="woall", name="woall")

            CW = KT * 394  # 2364 cols per x chunk
            HW = CW // 2   # half chunk = k0-2 or k3-5

            def dma_slice(eng, dst, src, a, b):
                eng.dma_start(dst[:, a:b], src[:, a:b])

            # head schedule built around measured per-queue DMA rates
            # (gpsimd ~166GB/s, scalar ~110, sync ~50): critical bytes on
            # the fast queues in consumption order, late-needed slabs on
            # sync. x chunk c: cols [c*CW + k*394 ...]; wqk pair p: cols
            # [p*1536 ...] (= m blocks 2p, 2p+1).
            W2 = 2 * D  # 1536 cols per wqk pair

            def wqk_slice(eng, a, b):
                dma_slice(eng, wqk_all, wqkd, a, b)

            def x_slice(eng, c, k0, k1):
                dma_slice(nc.__getattribute__(eng) if isinstance(eng, str) else eng,
                          x_all, xTd, c * CW + k0 * 394, c * CW + k1 * 394)

            wqk_slice(nc.scalar, 0, D)            # m0 first on scalar
            wqk_slice(nc.sync, D, 2 * D)          # m1 on sync (slow, small)
            for c in range(4):
                x_slice(nc.scalar, c, 0, 2)       # k0-1 on scalar
                x_slice(nc.gpsimd, c, 2, KT)      # k2-5 on gpsimd
            wqk_slice(nc.gpsimd, 1 * W2, 2 * W2)  # p1 right after x on gpsimd
            wqk_slice(nc.gpsimd, 2 * W2, 3 * W2)  # p2
            wqk_slice(nc.scalar, 3 * W2, 4 * W2)  # p3
            wqk_slice(nc.sync, 4 * W2, 5 * W2)    # p4
            wqk_slice(nc.sync, 5 * W2, 6 * W2)    # p5
            nc.sync.dma_start(bias_sb[:], bias)
            # wo on gpsimd (needed ~halfway), wv split scalar/gpsimd
            nc.gpsimd.dma_start(wo_all[:], wod)
            dma_slice(nc.scalar, wv_all, wvd, 0, KT * D // 2)
            dma_slice(nc.gpsimd, wv_all, wvd, KT * D // 2, KT * D)

            def x_ap(k, t0, tl):
                c, off = divmod(t0, 394)
                base = c * CW + k * 394 + off
                return x_all[:, base : base + tl]

            def wqk_ap(k, m):
                # m: use-order index; 2p = q pair p, 2p+1 = k pair p
                c = m * D + k * P
                return wqk_all[:, c : c + P]

            def wv_ap(k, c0, cl):
                return wv_all[:, k * D + c0 : k * D + c0 + cl]

            def wo_ap(k, c0, cl):
                return wo_all[:, k * D + c0 : k * D + c0 + cl]

            # qk_sb[m]: m<6 -> q head-pair m ; m>=6 -> k head-pair m-6.
            # layout [e within pair (2 heads x 64), t global]
            qk_sb = [big.tile([P, T], BF, tag=f"qk{m}", name=f"qk{m}") for m in range(2 * NPAIR)]
            # v tiles per (batch, j-tile): [j, 12*(64+1)] with ones columns
            v_sb = [big.tile([P, VW], BF, tag=f"v{i}", name=f"v{i}") for i in range(2 * BPC)]
            for i in range(2 * BPC):
                ones_cols = v_sb[i][:].rearrange("p (h c) -> p h c", c=DH + 1)[
                    :, :, DH : DH + 1
                ]
                nc.gpsimd.memset(ones_cols, 1.0)
            # attention output, [e, t] layout, tiles per (pair, batch-pair)
            at_sb = [
                [big.tile([P, N2], BF, tag=f"at{p}_{b2}", name=f"at{p}_{b2}") for b2 in range(BPC // 2)]
                for p in range(NPAIR)
            ]

            # ---- QK projection: qkT[e, t] = (w_qk x^T) ------------------
            # wave order per pair: (q,c0),(k,c0),(q,c1),(k,c1),... so the
            # head consumes each x chunk twice before needing the next —
            # halves the startup DMA demand rate.
            qk_alt = [0]
            for p in range(NPAIR):
                for t0, tl in T_CHUNKS:
                    for tgt, m in ((p, 2 * p), (NPAIR + p, 2 * p + 1)):
                        qk_alt[0] += 1
                        if qk_alt[0] % 5 < 3:
                            psum = ps_sc.tile([P, N2], F32, tag="sc", name="qksc")[:, :tl]
                        else:
                            psum = ps_pj.tile([P, 512], F32, tag="pj", name="pj")[:, :tl]
                        for k in range(KT):
                            nc.tensor.matmul(
                                psum,
                                wqk_ap(k, m),
                                x_ap(k, t0, tl),
                                start=(k == 0),
                                stop=(k == KT - 1),
                            )
                        nc.vector.tensor_copy(
                            out=qk_sb[tgt][:, t0 : t0 + tl], in_=psum
                        )

            # ---- V projection units (filler-interleaved) ----------------
            def vproj_unit(b, jt, c0, cl):
                def emit():
                    r0 = b * N + jt * P
                    rl = P if jt == 0 else JT1
                    i = 2 * b + jt
                    psum = ps_pj.tile([P, 512], F32, tag="pj", name="pjv")[:rl, :cl]
                    for k in range(KT):
                        nc.tensor.matmul(
                            psum,
                            x_ap(k, r0, rl),
                            wv_ap(k, c0, cl),
                            start=(k == 0),
                            stop=(k == KT - 1),
                        )
                    hs = c0 // DH
                    nh = cl // DH
                    out_ap = v_sb[i][
                        :rl, (DH + 1) * hs : (DH + 1) * (hs + nh)
                    ].rearrange("p (h c) -> p h c", c=DH + 1)[:, :, 0:DH]
                    nc.scalar.copy(
                        out=out_ap,
                        in_=psum.rearrange("p (h c) -> p h c", c=DH),
                    )

                return emit

            def vproj_units(b):
                return [
                    vproj_unit(b, jt, c0, cl)
                    for jt in range(2)
                    for c0, cl in ((0, 512), (512, 256))
                ]

            # ---- out-projection units -----------------------------------
            op_alt = [0]
            out_engs = [nc.sync, nc.scalar, nc.gpsimd]

            def outproj_unit(b2, m, vec=False, c0=0, cl=N2):
                def emit():
                    t0 = b2 * N2 + c0
                    op_alt[0] += 1
                    if op_alt[0] % 2 == 0:
                        psum = ps_sc.tile([P, N2], F32, tag="sc", name="opsc")[:, :cl]
                    else:
                        psum = ps_pj.tile([P, 512], F32, tag="pj", name="pjo")[:, :cl]
                    for k in range(KT):
                        nc.tensor.matmul(
                            psum,
                            wo_ap(k, m * P, P),
                            at_sb[k][b2][:, c0 : c0 + cl],
                            start=(k == 0),
                            stop=(k == KT - 1),
                        )
                    osb = sb_osb.tile([P, 512], BF, tag="osb", name="osb")[:, :cl]
                    if vec:
                        # final flush: vector is idle once the last pairs
                        # are done, scalar is the serial bottleneck there
                        nc.vector.tensor_scalar_add(osb, psum, bias_sb[:, m : m + 1])
                    else:
                        nc.scalar.activation(
                            osb,
                            psum,
                            mybir.ActivationFunctionType.Identity,
                            bias=bias_sb[:, m : m + 1],
                        )
                    out_engs[op_alt[0] % 3].dma_start(
                        outT[m * P : (m + 1) * P, t0 : t0 + cl], osb
                    )

                return emit

            # ---- one attention head-pair --------------------------------
            def emit_pair(b, p):
                tb = b * N
                qT = qk_sb[p]
                kTt = qk_sb[NPAIR + p]
                expT = []
                for h in (0, 1):
                    e0 = 64 * h
                    ps_s = ps_sc.tile([P, N2], F32, tag="sc", name="sc")
                    nc.tensor.matmul(
                        ps_s[0:P, 0:N],
                        kTt[e0 : e0 + DH, tb : tb + P],
                        qT[e0 : e0 + DH, tb : tb + N],
                        start=True,
                        stop=True,
                        tile_position=(e0, 0),
                    )
                    nc.tensor.matmul(
                        ps_s[0:JT1, N:N2],
                        kTt[e0 : e0 + DH, tb + P : tb + N],
                        qT[e0 : e0 + DH, tb : tb + N],
                        start=True,
                        stop=True,
                        tile_position=(e0, 0),
                    )
                    e = sb_exp.tile([P, N2], BF, tag="expT", name="expT")
                    nc.scalar.activation(e[:], ps_s[:], EXP)
                    expT.append(e)
                pso = ps_o.tile([DH + 1, N2], F32, tag="o", name="o")
                for h in (0, 1):
                    g = 2 * p + h
                    vc = (DH + 1) * g
                    nc.tensor.matmul(
                        pso[:, N * h : N * h + N],
                        v_sb[2 * b][0:P, vc : vc + DH + 1],
                        expT[h][0:P, 0:N],
                        start=True,
                        stop=False,
                    )
                    nc.tensor.matmul(
                        pso[:, N * h : N * h + N],
                        v_sb[2 * b + 1][0:JT1, vc : vc + DH + 1],
                        expT[h][0:JT1, N:N2],
                        start=False,
                        stop=True,
                    )
                # S row -> SBUF (base 0: custom DVE/GpSimd ops require it),
                # approx reciprocal, GpSimd partition broadcast, normalize
                # straight out of PSUM (single PSUM operand per DVE op);
                # the two muls split across vector/gpsimd to balance rates.
                s_sb = sb_rec.tile([1, N2], F32, tag="s_sb", name="s_sb")
                nc.vector.tensor_copy(out=s_sb[:], in_=pso[DH : DH + 1, :])
                rec = sb_rec.tile([1, N2], F32, tag="rec", name="rec")
                nc.vector.reciprocal_approx_fast(out=rec[:], in_=s_sb[:])
                bsb = sb_bsb.tile([DH, N2], F32, tag="bsb", name="bsb")
                nc.gpsimd.partition_broadcast(bsb[:], rec[:])
                for h in (0, 1):
                    nc.vector.tensor_mul(
                        out=at_sb[p][b // 2][
                            64 * h : 64 * h + DH, N * (b % 2) : N * (b % 2) + N
                        ],
                        in0=pso[0:DH, N * h : N * h + N],
                        in1=bsb[:, N * h : N * h + N],
                    )

            # ---- driver: attention with 1:1 projection filler -----------
            from collections import deque

            filler = deque()  # items: (kind, batch, emit_fn)
            for u in vproj_units(0) + vproj_units(1):
                u()
            filler.extend(("v", 2, u) for u in vproj_units(2))
            for b in range(BPC):
                # v tiles for batch b must be traced before its pairs
                for item in [it for it in filler if it[0] == "v" and it[1] <= b]:
                    filler.remove(item)
                    item[2]()
                for p in range(NPAIR):
                    emit_pair(b, p)
                    if filler:
                        filler.popleft()[2]()
                if b + 3 < BPC:
                    filler.extend(("v", b + 3, u) for u in vproj_units(b + 3))
                if b % 2 == 1 and b < 7:
                    filler.extend(
                        ("o", b, outproj_unit(b // 2, m)) for m in range(KT)
                    )
                if b == 6:
                    # batch-6 half of the last out-projection can flush
                    # during batch 7's pairs
                    filler.extend(
                        ("o", b, outproj_unit(3, m, c0=0, cl=N))
                        for m in range(KT)
                    )
                if b == 7:
                    filler.extend(
                        ("o", b, outproj_unit(3, m, vec=(m % 2 == 1), c0=N, cl=N))
                        for m in range(KT)
                    )
            while filler:
                filler.popleft()[2]()

    nc.compile()
    return nc


def host_in_maps(x, w_qkv, w_out, b_out):
    """Full fp32 inputs -> list of 8 per-core input dicts (bf16).

    Tensors are packed into the kernel's SBUF image (see build_nc):
      xTd  [128, 4*2364]: col = chunk*2364 + k*394 + t_off
      wqkd [128, 12*768]: col = m*768 + k*128 + c, m = q_p0,k_p0,q_p1,...
      wvd/wod [128, 6*768]: col = k*768 + c
    """
    bf16 = ml_dtypes.bfloat16
    wq = w_qkv[0:D] * SCALE
    wk = w_qkv[D : 2 * D]
    wv = w_qkv[2 * D : 3 * D]

    # wqkd: m-block = (q or k) rows [128p:128p+128]; transpose to [D, 128],
    # split D into k-tiles -> [k][128(p), 128] -> [p, k*128]
    blocks = []
    for p in range(NPAIR):
        for w in (wq, wk):
            blk = w[128 * p : 128 * (p + 1)].T  # [D, 128]
            blk = blk.reshape(KT, P, P).transpose(1, 0, 2).reshape(P, KT * P)
            blocks.append(blk)
    wqkd = np.ascontiguousarray(np.concatenate(blocks, axis=1)).astype(bf16)

    def pack_w(w):  # w [inner(D_in rows=e?), ...] -> [p, k*768]
        wT = w.T  # [D_in, D_out] with D_in = contraction
        return np.ascontiguousarray(
            wT.reshape(KT, P, D).transpose(1, 0, 2).reshape(P, KT * D)
        ).astype(bf16)

    wvd = pack_w(wv)
    wod = pack_w(w_out)
    bias = np.ascontiguousarray(b_out.reshape(KT, P).T).astype(np.float32)
    in_maps = []
    for c in range(NCORES):
        xc = x[c * BPC : (c + 1) * BPC].reshape(T, D)
        xT = xc.T  # [D, T]
        # [k, p, chunk, off] -> [p, chunk, k, off] -> [128, 4*2364]
        xTd = np.ascontiguousarray(
            xT.reshape(KT, P, 4, 394).transpose(1, 2, 0, 3).reshape(P, 4 * KT * 394)
        ).astype(bf16)
        in_maps.append(
            {"xTd": xTd, "wqkd": wqkd, "wvd": wvd, "wod": wod, "bias": bias}
        )
    return in_maps


def host_gather(results):
    """8 per-core {outT: [768, 1576] bf16} -> full [64, 197, 768] fp32."""
    out = np.empty((B, N, D), dtype=np.float32)
    for c in range(NCORES):
        oc = np.asarray(results[c]["outT"], dtype=np.float32)  # [D, T]
        out[c * BPC : (c + 1) * BPC] = oc.T.reshape(BPC, N, D)
    return out



_NC_CACHE = []


def kernel(x, w_qkv, w_out, b_out):
    """Full-input entry point: shards batch over 8 NeuronCores, runs the
    Bass kernel, gathers the full [64, 197, 768] fp32 output."""
    if not _NC_CACHE:
        _NC_CACHE.append(build_nc())
    nc = _NC_CACHE[0]
    in_maps = host_in_maps(
        np.asarray(x, dtype=np.float32),
        np.asarray(w_qkv, dtype=np.float32),
        np.asarray(w_out, dtype=np.float32),
        np.asarray(b_out, dtype=np.float32),
    )
    res = run_bass_kernel_spmd(nc, in_maps, core_ids=list(range(NCORES)))
    return host_gather(res.results)


# revision 34
# speedup vs baseline: 1.1049x; 1.0054x over previous
"""Builder + host glue for the ViT attention kernel on 8 trn2 cores.

Reference computation (per batch b):
    qkv = x @ w_qkv.T ; q,k,v split; per head: softmax(q k^T / sqrt(dh)) v
    out = attn @ w_out.T + b_out

Sharding: data-parallel over batch (8 batches per core).

Host-side the q/k weight columns are interleaved per head-pair
(q_p0 | k_p0 | q_p1 | k_p1 | ...) so the weight DMAs stream in exactly
the order the QK projection consumes them.
"""

import numpy as np
import ml_dtypes

import concourse.bass as bass
import concourse.tile as tile
from concourse import bacc, mybir
from concourse.bass_utils import run_bass_kernel_spmd

P = 128
B, N, D = 64, 197, 768
H, DH = 12, 64
NCORES = 8
BPC = B // NCORES          # 8 batches per core
T = BPC * N                # 1576 tokens per core
KT = D // P                # 6 contraction tiles
NPAIR = H // 2             # 6 head pairs
SCALE = DH ** -0.5
VW = (DH + 1) * H          # 780: v columns incl per-head ones column
N2 = 2 * N                 # 394
JT1 = N - P                # 69: second j-tile size

BF = mybir.dt.bfloat16
F32 = mybir.dt.float32
EXP = mybir.ActivationFunctionType.Exp

T_CHUNKS = [(0, 394), (394, 394), (788, 394), (1182, 394)]


def build_nc():
    nc = bacc.Bacc(
        "TRN2", target_bir_lowering=False, debug=False, num_devices=NCORES
    )
    # Inputs are host-packed into the exact SBUF image so every input DMA
    # is a contiguous 2D slice with multi-KB per-partition lines:
    #   xTd  [128, 4*2364]: col = chunk*2364 + k*394 + t_off
    #   wqkd [128, 12*768]: col = m*768 + k*128 + c  (m use-order: q_p0,k_p0,..)
    #   wvd/wod [128, 6*768]: col = k*768 + c
    xTd = nc.dram_tensor("xTd", [P, 4 * KT * 394], BF, kind="ExternalInput").ap()
    wqkd = nc.dram_tensor("wqkd", [P, 2 * NPAIR * D], BF, kind="ExternalInput").ap()
    wvd = nc.dram_tensor("wvd", [P, KT * D], BF, kind="ExternalInput").ap()
    wod = nc.dram_tensor("wod", [P, KT * D], BF, kind="ExternalInput").ap()
    bias = nc.dram_tensor("bias", [P, KT], F32, kind="ExternalInput").ap()
    outT = nc.dram_tensor("outT", [D, T], BF, kind="ExternalOutput").ap()

    with tile.TileContext(nc) as tc:
        with (
            tc.tile_pool(name="big", bufs=1) as big,
            tc.tile_pool(name="exp", bufs=12) as sb_exp,
            tc.tile_pool(name="rec", bufs=8) as sb_rec,
            tc.tile_pool(name="bsb", bufs=8) as sb_bsb,
            tc.tile_pool(name="osb", bufs=3) as sb_osb,
            tc.tile_pool(name="ps_pj", bufs=2, space="PSUM") as ps_pj,
            tc.tile_pool(name="ps_sc", bufs=3, space="PSUM") as ps_sc,
            tc.tile_pool(name="ps_o", bufs=3, space="PSUM") as ps_o,
        ):
            # ---- persistent buffers + input DMAs -------------------------
            # DMA throughput is line-size bound (~constant lines/us per
            # queue) and issue instructions cost ~650ns of engine time, so
            # each input DMA is a contiguous slice with a 1.5-4.7KB
            # per-partition line. Only sync/scalar/gpsimd can issue DMAs.
            bias_sb = big.tile([P, KT], F32, tag="bias")

            x_all = big.tile([P, 4 * KT * 394], BF, tag="xall", name="xall")
            wqk_all = big.tile([P, 2 * NPAIR * D], BF, tag="wqkall", name="wqkall")
            wv_all = big.tile([P, KT * D], BF, tag="wvall", name="wvall")
            wo_all = big.tile([P, KT * D], BF, tag="woall", name="woall")

            CW = KT * 394  # 2364 cols per x chunk
            HW = CW // 2   # half chunk = k0-2 or k3-5

            def dma_slice(eng, dst, src, a, b):
                eng.dma_start(dst[:, a:b], src[:, a:b])

            # head schedule built around measured per-queue DMA rates
            # (gpsimd ~166GB/s, scalar ~110, sync ~50): critical bytes on
            # the fast queues in consumption order, late-needed slabs on
            # sync. x chunk c: cols [c*CW + k*394 ...]; wqk pair p: cols
            # [p*1536 ...] (= m blocks 2p, 2p+1).
            W2 = 2 * D  # 1536 cols per wqk pair

            def wqk_slice(eng, a, b):
                dma_slice(eng, wqk_all, wqkd, a, b)

            def x_slice(eng, c, k0, k1):
                dma_slice(nc.__getattribute__(eng) if isinstance(eng, str) else eng,
                          x_all, xTd, c * CW + k0 * 394, c * CW + k1 * 394)

            # Throughput comes from several outstanding ~200-300KB DMAs per
            # queue spraying across the DMA engine pool — issue the whole
            # critical prefix up front, in consumption order.
            wqk_slice(nc.scalar, 0, D)            # m0
            wqk_slice(nc.sync, D, 2 * D)          # m1 on sync (small, early)
            for c in range(4):
                x_slice(nc.scalar, c, 0, 2)       # k0-1 on scalar
                x_slice(nc.gpsimd, c, 2, 4)       # k2-3 on gpsimd
                x_slice(nc.gpsimd, c, 4, KT)      # k4-5 on gpsimd
            wqk_slice(nc.gpsimd, 1 * W2, 2 * W2)  # p1
            wqk_slice(nc.gpsimd, 2 * W2, 3 * W2)  # p2
            wqk_slice(nc.scalar, 3 * W2, 4 * W2)  # p3
            wqk_slice(nc.sync, 4 * W2, 5 * W2)    # p4
            wqk_slice(nc.sync, 5 * W2, 6 * W2)    # p5
            nc.sync.dma_start(bias_sb[:], bias)
            # wo (needed ~halfway) and wv split to keep queues fed
            nc.gpsimd.dma_start(wo_all[:], wod)
            dma_slice(nc.scalar, wv_all, wvd, 0, KT * D // 2)
            dma_slice(nc.gpsimd, wv_all, wvd, KT * D // 2, KT * D)

            def x_ap(k, t0, tl):
                c, off = divmod(t0, 394)
                base = c * CW + k * 394 + off
                return x_all[:, base : base + tl]

            def wqk_ap(k, m):
                # m: use-order index; 2p = q pair p, 2p+1 = k pair p
                c = m * D + k * P
                return wqk_all[:, c : c + P]

            def wv_ap(k, c0, cl):
                return wv_all[:, k * D + c0 : k * D + c0 + cl]

            def wo_ap(k, c0, cl):
                return wo_all[:, k * D + c0 : k * D + c0 + cl]

            # qk_sb[m]: m<6 -> q head-pair m ; m>=6 -> k head-pair m-6.
            # layout [e within pair (2 heads x 64), t global]
            qk_sb = [big.tile([P, T], BF, tag=f"qk{m}", name=f"qk{m}") for m in range(2 * NPAIR)]
            # v tiles per (batch, j-tile): [j, 12*(64+1)] with ones columns
            v_sb = [big.tile([P, VW], BF, tag=f"v{i}", name=f"v{i}") for i in range(2 * BPC)]
            for i in range(2 * BPC):
                ones_cols = v_sb[i][:].rearrange("p (h c) -> p h c", c=DH + 1)[
                    :, :, DH : DH + 1
                ]
                nc.gpsimd.memset(ones_cols, 1.0)
            # attention output, [e, t] layout, tiles per (pair, batch-pair)
            at_sb = [
                [big.tile([P, N2], BF, tag=f"at{p}_{b2}", name=f"at{p}_{b2}") for b2 in range(BPC // 2)]
                for p in range(NPAIR)
            ]

            # ---- QK projection: qkT[e, t] = (w_qk x^T) ------------------
            # wave order per pair: (q,c0),(k,c0),(q,c1),(k,c1),... so the
            # head consumes each x chunk twice before needing the next —
            # halves the startup DMA demand rate.
            qk_alt = [0]
            for p in range(NPAIR):
                for t0, tl in T_CHUNKS:
                    for tgt, m in ((p, 2 * p), (NPAIR + p, 2 * p + 1)):
                        qk_alt[0] += 1
                        if qk_alt[0] % 5 < 3:
                            psum = ps_sc.tile([P, N2], F32, tag="sc", name="qksc")[:, :tl]
                        else:
                            psum = ps_pj.tile([P, 512], F32, tag="pj", name="pj")[:, :tl]
                        for k in range(KT):
                            nc.tensor.matmul(
                                psum,
                                wqk_ap(k, m),
                                x_ap(k, t0, tl),
                                start=(k == 0),
                                stop=(k == KT - 1),
                            )
                        nc.vector.tensor_copy(
                            out=qk_sb[tgt][:, t0 : t0 + tl], in_=psum
                        )

            # ---- V projection units (filler-interleaved) ----------------
            def vproj_unit(b, jt, c0, cl):
                def emit():
                    r0 = b * N + jt * P
                    rl = P if jt == 0 else JT1
                    i = 2 * b + jt
                    psum = ps_pj.tile([P, 512], F32, tag="pj", name="pjv")[:rl, :cl]
                    for k in range(KT):
                        nc.tensor.matmul(
                            psum,
                            x_ap(k, r0, rl),
                            wv_ap(k, c0, cl),
                            start=(k == 0),
                            stop=(k == KT - 1),
                        )
                    hs = c0 // DH
                    nh = cl // DH
                    out_ap = v_sb[i][
                        :rl, (DH + 1) * hs : (DH + 1) * (hs + nh)
                    ].rearrange("p (h c) -> p h c", c=DH + 1)[:, :, 0:DH]
                    nc.scalar.copy(
                        out=out_ap,
                        in_=psum.rearrange("p (h c) -> p h c", c=DH),
                    )

                return emit

            def vproj_units(b):
                return [
                    vproj_unit(b, jt, c0, cl)
                    for jt in range(2)
                    for c0, cl in ((0, 512), (512, 256))
                ]

            # ---- out-projection units -----------------------------------
            op_alt = [0]
            out_engs = [nc.sync, nc.scalar, nc.gpsimd]

            def outproj_unit(b2, m, vec=False, c0=0, cl=N2):
                def emit():
                    t0 = b2 * N2 + c0
                    op_alt[0] += 1
                    if op_alt[0] % 2 == 0:
                        psum = ps_sc.tile([P, N2], F32, tag="sc", name="opsc")[:, :cl]
                    else:
                        psum = ps_pj.tile([P, 512], F32, tag="pj", name="pjo")[:, :cl]
                    for k in range(KT):
                        nc.tensor.matmul(
                            psum,
                            wo_ap(k, m * P, P),
                            at_sb[k][b2][:, c0 : c0 + cl],
                            start=(k == 0),
                            stop=(k == KT - 1),
                        )
                    osb = sb_osb.tile([P, 512], BF, tag="osb", name="osb")[:, :cl]
                    if vec:
                        # final flush: vector is idle once the last pairs
                        # are done, scalar is the serial bottleneck there
                        nc.vector.tensor_scalar_add(osb, psum, bias_sb[:, m : m + 1])
                    else:
                        nc.scalar.activation(
                            osb,
                            psum,
                            mybir.ActivationFunctionType.Identity,
                            bias=bias_sb[:, m : m + 1],
                        )
                    out_engs[op_alt[0] % 3].dma_start(
                        outT[m * P : (m + 1) * P, t0 : t0 + cl], osb
                    )

                return emit

            # ---- one attention head-pair --------------------------------
            def emit_pair(b, p):
                tb = b * N
                qT = qk_sb[p]
                kTt = qk_sb[NPAIR + p]
                expT = []
                for h in (0, 1):
                    e0 = 64 * h
                    ps_s = ps_sc.tile([P, N2], F32, tag="sc", name="sc")
                    nc.tensor.matmul(
                        ps_s[0:P, 0:N],
                        kTt[e0 : e0 + DH, tb : tb + P],
                        qT[e0 : e0 + DH, tb : tb + N],
                        start=True,
                        stop=True,
                        tile_position=(e0, 0),
                    )
                    nc.tensor.matmul(
                        ps_s[0:JT1, N:N2],
                        kTt[e0 : e0 + DH, tb + P : tb + N],
                        qT[e0 : e0 + DH, tb : tb + N],
                        start=True,
                        stop=True,
                        tile_position=(e0, 0),
                    )
                    e = sb_exp.tile([P, N2], BF, tag="expT", name="expT")
                    nc.scalar.activation(e[:], ps_s[:], EXP)
                    expT.append(e)
                pso = ps_o.tile([DH + 1, N2], F32, tag="o", name="o")
                for h in (0, 1):
                    g = 2 * p + h
                    vc = (DH + 1) * g
                    nc.tensor.matmul(
                        pso[:, N * h : N * h + N],
                        v_sb[2 * b][0:P, vc : vc + DH + 1],
                        expT[h][0:P, 0:N],
                        start=True,
                        stop=False,
                    )
                    nc.tensor.matmul(
                        pso[:, N * h : N * h + N],
                        v_sb[2 * b + 1][0:JT1, vc : vc + DH + 1],
                        expT[h][0:JT1, N:N2],
                        start=False,
                        stop=True,
                    )
                # S row -> SBUF (base 0: custom DVE/GpSimd ops require it),
                # approx reciprocal, GpSimd partition broadcast, normalize
                # straight out of PSUM (single PSUM operand per DVE op);
                # the two muls split across vector/gpsimd to balance rates.
                s_sb = sb_rec.tile([1, N2], F32, tag="s_sb", name="s_sb")
                nc.vector.tensor_copy(out=s_sb[:], in_=pso[DH : DH + 1, :])
                rec = sb_rec.tile([1, N2], F32, tag="rec", name="rec")
                nc.vector.reciprocal_approx_fast(out=rec[:], in_=s_sb[:])
                bsb = sb_bsb.tile([DH, N2], F32, tag="bsb", name="bsb")
                nc.gpsimd.partition_broadcast(bsb[:], rec[:])
                for h in (0, 1):
                    nc.vector.tensor_mul(
                        out=at_sb[p][b // 2][
                            64 * h : 64 * h + DH, N * (b % 2) : N * (b % 2) + N
                        ],
                        in0=pso[0:DH, N * h : N * h + N],
                        in1=bsb[:, N * h : N * h + N],
                    )

            # ---- driver: attention with 1:1 projection filler -----------
            from collections import deque

            filler = deque()  # items: (kind, batch, emit_fn)
            for u in vproj_units(0) + vproj_units(1):
                u()
            filler.extend(("v", 2, u) for u in vproj_units(2))
            for b in range(BPC):
                # v tiles for batch b must be traced before its pairs
                for item in [it for it in filler if it[0] == "v" and it[1] <= b]:
                    filler.remove(item)
                    item[2]()
                for p in range(NPAIR):
                    emit_pair(b, p)
                    if filler:
                        filler.popleft()[2]()
                if b + 3 < BPC:
                    filler.extend(("v", b + 3, u) for u in vproj_units(b + 3))
                if b % 2 == 1 and b < 7:
                    filler.extend(
                        ("o", b, outproj_unit(b // 2, m)) for m in range(KT)
                    )
                if b == 6:
                    # batch-6 half of the last out-projection can flush
                    # during batch 7's pairs
                    filler.extend(
                        ("o", b, outproj_unit(3, m, c0=0, cl=N))
                        for m in range(KT)
                    )
                if b == 7:
                    filler.extend(
                        ("o", b, outproj_unit(3, m, vec=(m % 2 == 1), c0=N, cl=N))
                        for m in range(KT)
                    )
            while filler:
                filler.popleft()[2]()

    nc.compile()
    return nc


def host_in_maps(x, w_qkv, w_out, b_out):
    """Full fp32 inputs -> list of 8 per-core input dicts (bf16).

    Tensors are packed into the kernel's SBUF image (see build_nc):
      xTd  [128, 4*2364]: col = chunk*2364 + k*394 + t_off
      wqkd [128, 12*768]: col = m*768 + k*128 + c, m = q_p0,k_p0,q_p1,...
      wvd/wod [128, 6*768]: col = k*768 + c
    """
    bf16 = ml_dtypes.bfloat16
    wq = w_qkv[0:D] * SCALE
    wk = w_qkv[D : 2 * D]
    wv = w_qkv[2 * D : 3 * D]

    # wqkd: m-block = (q or k) rows [128p:128p+128]; transpose to [D, 128],
    # split D into k-tiles -> [k][128(p), 128] -> [p, k*128]
    blocks = []
    for p in range(NPAIR):
        for w in (wq, wk):
            blk = w[128 * p : 128 * (p + 1)].T  # [D, 128]
            blk = blk.reshape(KT, P, P).transpose(1, 0, 2).reshape(P, KT * P)
            blocks.append(blk)
    wqkd = np.ascontiguousarray(np.concatenate(blocks, axis=1)).astype(bf16)

    def pack_w(w):  # w [inner(D_in rows=e?), ...] -> [p, k*768]
        wT = w.T  # [D_in, D_out] with D_in = contraction
        return np.ascontiguousarray(
            wT.reshape(KT, P, D).transpose(1, 0, 2).reshape(P, KT * D)
        ).astype(bf16)

    wvd = pack_w(wv)
    wod = pack_w(w_out)
    bias = np.ascontiguousarray(b_out.reshape(KT, P).T).astype(np.float32)
    in_maps = []
    for c in range(NCORES):
        xc = x[c * BPC : (c + 1) * BPC].reshape(T, D)
        xT = xc.T  # [D, T]
        # [k, p, chunk, off] -> [p, chunk, k, off] -> [128, 4*2364]
        xTd = np.ascontiguousarray(
            xT.reshape(KT, P, 4, 394).transpose(1, 2, 0, 3).reshape(P, 4 * KT * 394)
        ).astype(bf16)
        in_maps.append(
            {"xTd": xTd, "wqkd": wqkd, "wvd": wvd, "wod": wod, "bias": bias}
        )
    return in_maps


def host_gather(results):
    """8 per-core {outT: [768, 1576] bf16} -> full [64, 197, 768] fp32."""
    out = np.empty((B, N, D), dtype=np.float32)
    for c in range(NCORES):
        oc = np.asarray(results[c]["outT"], dtype=np.float32)  # [D, T]
        out[c * BPC : (c + 1) * BPC] = oc.T.reshape(BPC, N, D)
    return out



_NC_CACHE = []


def kernel(x, w_qkv, w_out, b_out):
    """Full-input entry point: shards batch over 8 NeuronCores, runs the
    Bass kernel, gathers the full [64, 197, 768] fp32 output."""
    if not _NC_CACHE:
        _NC_CACHE.append(build_nc())
    nc = _NC_CACHE[0]
    in_maps = host_in_maps(
        np.asarray(x, dtype=np.float32),
        np.asarray(w_qkv, dtype=np.float32),
        np.asarray(w_out, dtype=np.float32),
        np.asarray(b_out, dtype=np.float32),
    )
    res = run_bass_kernel_spmd(nc, in_maps, core_ids=list(range(NCORES)))
    return host_gather(res.results)
